# revision 51
# baseline (speedup 1.0000x reference)
"""Sharded causal-attention kernel for 8 trn2 NeuronCores.

DP over batch (2) x TP over head groups (4 heads/core). Each core: qkv projection
(its heads) + RoPE + causal SDPA (scores kept transposed; softmax denominator via a
ones-column in the PV matmul) + its 256-row slice of the o_proj contraction, returning
a transposed partial [HID, S]; the host sums 4 partials per batch. bf16 matmuls,
fp32 PSUM accumulation.

Schedule: PE warmup dummies bridge the DMA bring-up so real matmuls start at full
clock; o_proj of chunk qi is interleaved per-ki into chunk qi+1's attention so the
PE never stalls on the exp chain; softmax division reads PV numerators straight
from PSUM; score matmuls and exp are trimmed to live columns on the causal diagonal.
"""

import sys

sys.path.insert(0, "/opt/trn_rl_repo")

from contextlib import ExitStack

import numpy as np
import ml_dtypes

import concourse.bass as bass
import concourse.mybir as mybir
import concourse.tile as tile
from concourse import bacc

FP = mybir.dt.float32
BF = mybir.dt.bfloat16
EXP = mybir.ActivationFunctionType.Exp

B, S, HID = 2, 2048, 1024
H, D = 16, 64
QC = 512
KT = 128
NQC = S // QC
NKT = S // KT
KHID = HID // 128
N_DUMMY = 11


def build_program():
    nc = bacc.Bacc("TRN2", target_bir_lowering=False, debug=False, num_devices=8, num_swdge_queues=2)

    wqkT = nc.dram_tensor("wqkT", [128, KHID * 512], BF, kind="ExternalInput").ap()
    hsbT = nc.dram_tensor("hsbT", [128, NQC * KHID * QC], BF, kind="ExternalInput").ap()
    wvT = nc.dram_tensor("wvT", [128, KHID * 256], BF, kind="ExternalInput").ap()
    woT = nc.dram_tensor("woT", [128, 2 * HID], BF, kind="ExternalInput").ap()
    cos2T = nc.dram_tensor("cos2T", [64, S], BF, kind="ExternalInput").ap()
    ssin2T = nc.dram_tensor("ssin2T", [64, S], BF, kind="ExternalInput").ap()
    maskD = nc.dram_tensor("maskD", [128, 256], BF, kind="ExternalInput").ap()
    pmat = nc.dram_tensor("pmat", [128, 128], BF, kind="ExternalInput").ap()
    outT = nc.dram_tensor("outT", [128, NQC * 8 * QC], BF, kind="ExternalOutput").ap()

    with tile.TileContext(nc) as tc:
        build_tile_program(tc, wqkT, hsbT, wvT, woT, cos2T, ssin2T, maskD, pmat, outT)
    nc.compile()
    return nc


def build_tile_program(tc, wqkT, hsbT, wvT, woT, cos2T, ssin2T, maskD, pmat, outT):
    nc = tc.nc
    with ExitStack() as ctx:
        const = ctx.enter_context(tc.tile_pool(name="const", bufs=1))
        persist = ctx.enter_context(tc.tile_pool(name="persist", bufs=1))
        work = ctx.enter_context(tc.tile_pool(name="work", bufs=5))
        owp = ctx.enter_context(tc.tile_pool(name="owp", bufs=2))
        expp = ctx.enter_context(tc.tile_pool(name="expp", bufs=8))
        small = ctx.enter_context(tc.tile_pool(name="small", bufs=3))
        ps_main = ctx.enter_context(tc.tile_pool(name="ps_main", bufs=2, space="PSUM"))
        ps_sc = ctx.enter_context(tc.tile_pool(name="ps_sc", bufs=2, space="PSUM"))

        # ---- PE warmup scratch (no DMA dependency) ----
        dum_l = const.tile([128, 128], BF, name="dum_l")
        dum_r = const.tile([128, QC], BF, name="dum_r")
        nc.gpsimd.memset(dum_l[:], 0.0)
        nc.vector.memset(dum_r[:], 0.0)

        # ---- input DMAs, in three dependency-gated stages so the transfers
        # needed first get the full DMA bandwidth instead of a fair share ----
        wqk_sb = const.tile([128, KHID, 512], BF, name="wqk_sb")
        hsb_sb = const.tile([128, NQC, KHID, QC], BF, name="hsb_sb")
        cos_sb = const.tile([128, S], BF, name="cos_sb")
        ssin_sb = const.tile([128, S], BF, name="ssin_sb")
        pmat_sb = const.tile([128, 128], BF, name="pmat_sb")
        tri_sb = const.tile([128, 2, 128], BF, name="tri_sb")
        wv_sb = const.tile([128, KHID, 256], BF, name="wv_sb")
        wo_sb = const.tile([128, 2, HID], BF, name="wo_sb")

        wqkR = wqkT.rearrange("p (k m) -> p k m", k=KHID)

        def load_wqk_rb(rb, eng):
            eng.dma_start(
                wqk_sb[:, :, rb * 128:(rb + 1) * 128], wqkR[:, :, rb * 128:(rb + 1) * 128]
            )

        def load_hsb(t, eng):
            eng.dma_start(
                hsb_sb[:, t],
                hsbT[:, t * KHID * QC:(t + 1) * KHID * QC].rearrange("p (k s) -> p k s", k=KHID),
            )

        # stage 0: everything the first proj/rope of chunk 0 (pair0) touches,
        # split fine so the descriptors spread across many DMA queues
        load_wqk_rb(0, nc.sync)
        for kk in range(4):
            eng = nc.sync if kk % 2 == 0 else nc.scalar
            eng.dma_start(
                hsb_sb[:, 0, 2 * kk:2 * kk + 2, :],
                hsbT[:, 2 * kk * QC:(2 * kk + 2) * QC].rearrange("p (k s) -> p k s", k=2),
            )
        nc.scalar.dma_start(cos_sb[0:64, :], cos2T[:])
        nc.sync.dma_start(ssin_sb[0:64, :], ssin2T[:])
        nc.scalar.dma_start(pmat_sb[:], pmat[:])
        load_wqk_rb(2, nc.scalar)

        # stage gates: 2-byte copies whose WAW deps hold the next stage's DMAs
        # back until the anchor transfer of the prior stage has completed
        anchor0 = hsb_sb[0:1, 0, 7, 0:1]
        for dst in (wqk_sb[0:1, 0, 128:129], wqk_sb[0:1, 0, 384:385],
                    wv_sb[0:1, 0, 0:1], tri_sb[0:1, 0, 0:1],
                    hsb_sb[0:1, 1, 0, 0:1]):
            nc.vector.tensor_copy(dst, anchor0)
        # stage 1: pair1 weights, V weights, mask, chunk 1
        load_wqk_rb(1, nc.sync)
        for kk in range(2):
            eng = nc.sync if kk == 0 else nc.scalar
            eng.dma_start(
                hsb_sb[:, 1, 4 * kk:4 * kk + 4, :],
                hsbT[:, (8 + 4 * kk) * QC:(8 + 4 * kk + 4) * QC].rearrange(
                    "p (k s) -> p k s", k=4),
            )
        nc.scalar.dma_start(wv_sb[:], wvT.rearrange("p (k m) -> p k m", k=KHID))
        nc.sync.dma_start(tri_sb[:], maskD.rearrange("p (r c) -> p r c", r=2))
        load_wqk_rb(3, nc.scalar)

        anchor1 = hsb_sb[0:1, 1, 7, 0:1]
        for dst in (wo_sb[0:1, 0, 0:1], hsb_sb[0:1, 2, 0, 0:1],
                    hsb_sb[0:1, 3, 0, 0:1]):
            nc.vector.tensor_copy(dst, anchor1)
        # stage 2: o_proj weights, chunks 2-3
        nc.scalar.dma_start(wo_sb[:], woT.rearrange("p (k m) -> p k m", k=2))
        load_hsb(2, nc.sync)
        load_hsb(3, nc.scalar)

        nc.vector.tensor_copy(cos_sb[64:128, :], cos_sb[0:64, :])
        nc.vector.tensor_copy(ssin_sb[64:128, :], ssin_sb[0:64, :])

        qkT = persist.tile([128, 4, S], BF, name="qkT")
        l_tiles = persist.tile([64, 8, QC], FP, name="l_tiles")
        nc.gpsimd.memset(l_tiles[:], 1.0)
        v_sb = persist.tile([128, NKT, 4 * 65], BF, name="v_sb2")
        att_sb = persist.tile([128, 2, S], BF, name="att_sb2")
        nc.gpsimd.memset(
            v_sb.rearrange("p t (h c) -> p t h c", c=65)[:, :, :, 64:65], 1.0
        )

        # ---- PE warmup: keep the array streaming until real inputs land ----
        for _ in range(N_DUMMY):
            psd = ps_sc.tile([128, QC], FP, name="psd", tag="sc")
            nc.tensor.matmul(psd[:], dum_l[:], dum_r[:], start=True, stop=True)

        def proj_mm(rb, t):
            """bf16 Q/K projection matmuls; returns (psum tile, x bf16 copy)."""
            ps = ps_main.tile([128, QC], FP, name="ps_qk", tag="ps")
            for kk in range(KHID):
                nc.tensor.matmul(
                    ps[:],
                    wqk_sb[:, kk, rb * 128:(rb + 1) * 128],
                    hsb_sb[:, t, kk, :],
                    start=(kk == 0),
                    stop=(kk == KHID - 1),
                )
            x = work.tile([128, QC], BF, name="x_rope", tag="xrope")
            nc.scalar.copy(x[:], ps[:])
            return ps, x

        def rope_mm(x):
            xs_ps = ps_main.tile([128, QC], FP, name="xs_ps", tag="ps")
            nc.tensor.matmul(xs_ps[:], pmat_sb[:], x[:], start=True, stop=True)
            return xs_ps

        def rope_fin(rb, t, x, xs_ps):
            csl = slice(t * QC, (t + 1) * QC)
            t1 = work.tile([128, QC], BF, name="t1_rope", tag="t1rope")
            t2 = work.tile([128, QC], BF, name="t2_rope", tag="t2rope")
            nc.vector.tensor_mul(t1[:], x[:], cos_sb[:, csl])
            nc.vector.tensor_mul(t2[:], xs_ps[:], ssin_sb[:, csl])
            nc.vector.tensor_add(qkT[:, rb, csl], t1[:], t2[:])

        def v_proj(tt):
            psv = ps_main.tile([128, 256], FP, name="ps_v", tag="ps")
            for kk in range(KHID):
                nc.tensor.matmul(
                    psv[:],
                    hsb_sb[:, tt // 4, kk, (tt % 4) * 128:(tt % 4 + 1) * 128],
                    wv_sb[:, kk, :],
                    start=(kk == 0),
                    stop=(kk == KHID - 1),
                )
            nc.vector.tensor_copy(
                v_sb[:, tt, :].rearrange("p (h c) -> p h c", c=65)[:, :, 0:64],
                psv[:].rearrange("p (h c) -> p h c", c=64),
            )

        def proj_block(t):
            """Full projection of chunk t: Q/K (pair0 rbs first) + V."""
            parts = {}
            for rb in (0, 2, 1, 3):
                parts[rb] = proj_mm(rb, t)
            for rb in (0, 2, 1, 3):
                ps, x = parts[rb]
                xs_ps = rope_mm(x)
                rope_fin(rb, t, x, xs_ps)
            for tt in range(4 * t, 4 * t + 4):
                v_proj(tt)

        def attention_unit(pair, qi, l_pair, fillers, max_pops=99, l_on_act=False):
            """scores^T -> exp -> PV for heads (2*pair, 2*pair+1); po kept in PSUM."""
            nki = 4 * qi + 4
            pops = 0
            po0 = ps_main.tile([65, QC], FP, name="po0", tag="po", bufs=2)
            po1 = ps_main.tile([65, QC], FP, name="po1", tag="po", bufs=2)
            for ki in range(nki):
                ksl = slice(ki * KT, (ki + 1) * KT)
                j = ki - 4 * qi
                lo = 0 if j < 0 else 128 * j  # first live q column in this chunk
                qsl = slice(qi * QC + lo, (qi + 1) * QC)
                psc = ps_sc.tile([128, 2, QC], FP, name="psc", tag="sc")
                nc.tensor.matmul(
                    psc[:, 0, lo:QC], qkT[0:64, 2 + pair, ksl], qkT[0:64, pair, qsl],
                    start=True, stop=True,
                )
                nc.tensor.matmul(
                    psc[:, 1, lo:QC], qkT[64:128, 2 + pair, ksl], qkT[64:128, pair, qsl],
                    start=True, stop=True,
                )
                e = expp.tile([128, 2, QC], BF, name="e", tag="exp")
                nc.scalar.activation(
                    e[:, :, lo:QC], psc[:, :, lo:QC], EXP, scale=0.125
                )
                if j >= 0:
                    nc.vector.tensor_mul(
                        e[:, :, lo:lo + 128], e[:, :, lo:lo + 128], tri_sb[:]
                    )
                h0 = 2 * pair
                h1 = 2 * pair + 1
                nc.tensor.matmul(
                    po0[:, lo:QC], v_sb[:, ki, h0 * 65:(h0 + 1) * 65], e[:, 0, lo:QC],
                    start=(ki == 0), stop=(ki == nki - 1),
                )
                nc.tensor.matmul(
                    po1[:, lo:QC], v_sb[:, ki, h1 * 65:(h1 + 1) * 65], e[:, 1, lo:QC],
                    start=(ki == 0), stop=(ki == nki - 1),
                )
                if (fillers and pops < max_pops
                        and (qi < 2 or ki % 2 == 0) and ki < nki - 2):
                    fillers.pop(0)()
                    pops += 1
            l_eng = nc.scalar.copy if l_on_act else nc.vector.tensor_copy
            l_eng(l_pair[0:1, :], po0[64:65, :])
            l_eng(l_pair[32:33, :], po1[64:65, :])
            return po0, po1

        def division(pair, qi, l_pair, po0, po1):
            qsl = slice(qi * QC, (qi + 1) * QC)
            rl = small.tile([64, QC], FP, name="rl", tag="rl", bufs=6)
            scr = small.tile([64, QC], FP, name="scr", tag="scr", bufs=3)
            nc.vector.reciprocal_approx_accurate(out=rl[:], in_=l_pair[:], scratch=scr[:])
            for sub, po in enumerate([po0, po1]):
                if sub == 0:
                    src = rl[0:1, :]
                else:
                    rlrow = small.tile([1, QC], FP, name="rlrow", tag="rlrow", bufs=4)
                    nc.vector.tensor_copy(rlrow[:], rl[32:33, :])
                    src = rlrow[:]
                rb_ = small.tile([64, QC], FP, name="rb_", tag="rbb", bufs=6)
                nc.gpsimd.partition_broadcast(rb_[:], src)
                nc.vector.tensor_mul(
                    att_sb[sub * 64:(sub + 1) * 64, pair, qsl], po[0:64, :], rb_[:]
                )

        def oproj_units(qi, tail=False):
            """Eight filler units (one per output row block) for o_proj of chunk qi."""
            qsl = slice(qi * QC, (qi + 1) * QC)

            def mk(ot):
                def f():
                    ow = owp.tile([128, QC], BF, name="ow", tag="ow", bufs=4)
                    pw = ps_main.tile([128, QC], FP, name="pw", tag="ps")
                    for p in range(2):
                        nc.tensor.matmul(
                            pw[:],
                            wo_sb[:, p, ot * 128:(ot + 1) * 128],
                            att_sb[:, p, qsl],
                            start=(p == 0),
                            stop=(p == 1),
                        )
                    if tail and ot % 2 == 1:
                        nc.scalar.copy(ow[:], pw[:])
                    else:
                        nc.vector.tensor_copy(ow[:], pw[:])
                    off = (qi * 8 + ot) * QC
                    nc.sync.dma_start(outT[:, off:off + QC], ow[:])
                return f

            return [mk(ot) for ot in range(8)]

        # ---- emission ----
        proj_block(0)
        fillers = []
        for qi in range(NQC):
            last = qi + 1 == NQC
            pops = 2 if last else 99
            l0 = l_tiles[:, 2 * qi, :]
            pa = attention_unit(0, qi, l0, fillers, max_pops=pops)
            division(0, qi, l0, *pa)
            l1 = l_tiles[:, 2 * qi + 1, :]
            pb = attention_unit(1, qi, l1, fillers, max_pops=pops, l_on_act=last)
            if not last:
                proj_block(qi + 1)
                division(1, qi, l1, *pb)
            else:
                # division first so its DVE chain runs under the drained
                # oproj(qi-1) matmuls, then the final chunk's o_proj
                division(1, qi, l1, *pb)
                for f in fillers:
                    f()
            fillers = oproj_units(qi, tail=last)
        for f in fillers:
            f()


# ---------- host-side shard preparation ----------

def make_core_inputs(hidden_states, cos, sin, w_qkv, w_o):
    """Returns list of 8 in_maps (numpy, bf16 where needed)."""
    bf = ml_dtypes.bfloat16
    hs = np.asarray(hidden_states, np.float32)
    cos = np.asarray(cos, np.float32)
    sin = np.asarray(sin, np.float32)
    w_qkv = np.asarray(w_qkv, np.float32)
    w_o = np.asarray(w_o, np.float32)

    cos2T = np.ascontiguousarray(cos.T).astype(bf)
    ssin2T = np.ascontiguousarray(sin.T).astype(bf)
    # signed rotate-half permutation: out[m] = sign(m) * x[partner(m)]
    # lhsT layout: pmat[k, m] = sign(m) at k = partner(m)
    pmat = np.zeros((128, 128), np.float32)
    for m in range(128):
        d = m % 64
        base = m - d
        if d < 32:
            pmat[base + d + 32, m] = -1.0
        else:
            pmat[base + d - 32, m] = 1.0
    pmat = pmat.astype(bf)

    kp = np.arange(128)[:, None]
    cc = np.arange(128)[None, :]
    tri = (kp <= cc).astype(bf)
    maskD = np.concatenate([tri, tri], axis=1)

    def swz(a):
        # [K*128, M] -> [128, K*M] partition-major (matches SBUF tiles)
        k = a.shape[0] // 128
        return np.ascontiguousarray(
            a.reshape(k, 128, a.shape[1]).transpose(1, 0, 2).reshape(128, -1))

    in_maps = []
    for c in range(8):
        b, g = divmod(c, 4)
        heads = range(4 * g, 4 * g + 4)
        hsT = np.ascontiguousarray(hs[b].T).astype(bf)  # [HID, S]
        # chunk-major swizzle: [128, t, kk, s']
        hsb = np.ascontiguousarray(
            hsT.reshape(8, 128, 4, 512).transpose(1, 2, 0, 3).reshape(128, -1))
        wq = np.concatenate([w_qkv[h * 64:(h + 1) * 64] for h in heads], 0)
        wk = np.concatenate([w_qkv[HID + h * 64:HID + (h + 1) * 64] for h in heads], 0)
        wv = np.concatenate([w_qkv[2 * HID + h * 64:2 * HID + (h + 1) * 64] for h in heads], 0)
        wqkT = swz(np.ascontiguousarray(np.concatenate([wq, wk], 0).T).astype(bf))
        wvT = swz(np.ascontiguousarray(wv.T).astype(bf))
        woT = swz(np.ascontiguousarray(
            np.concatenate([w_o[:, h * 64:(h + 1) * 64] for h in heads], 1).T
        ).astype(bf))
        in_maps.append({
            "hsbT": hsb, "wqkT": wqkT, "wvT": wvT, "woT": woT,
            "cos2T": cos2T, "ssin2T": ssin2T, "maskD": maskD, "pmat": pmat,
        })
    return in_maps


def unswizzle_out(o2):
    # [128, qi*half*oi*512] -> outT [1024, 2048]
    a = o2.reshape(128, NQC, 2, 4, QC)
    return np.ascontiguousarray(
        a.transpose(2, 3, 0, 1, 4).reshape(HID, S))


# ---------- standalone kernel entry ----------

from concourse.bass_utils import run_bass_kernel_spmd

_CACHED_NC = None


def get_program():
    global _CACHED_NC
    if _CACHED_NC is None:
        _CACHED_NC = build_program()
    return _CACHED_NC


def run(inputs, trace=False):
    nc = get_program()
    in_maps = make_core_inputs(**inputs)
    res = run_bass_kernel_spmd(nc, in_maps, core_ids=list(range(8)), trace=trace)
    out = np.zeros((B, S, HID), np.float32)
    for c, r in enumerate(res.results):
        out[c // 4] += unswizzle_out(r["outT"]).T.astype(np.float32)
    return out, res


def kernel(**inputs):
    out, _ = run(inputs, trace=False)
    return out


# revision 58
# speedup vs baseline: 1.0188x; 1.0188x over previous
"""Sharded causal-attention kernel for 8 trn2 NeuronCores.

DP over batch (2) x TP over head groups (4 heads/core). Each core: qkv projection
(its heads) + RoPE + causal SDPA (scores kept transposed; softmax denominator via a
ones-column in the PV matmul) + its 256-row slice of the o_proj contraction, returning
a transposed partial [HID, S]; the host sums 4 partials per batch. bf16 matmuls,
fp32 PSUM accumulation.

Schedule: PE warmup dummies bridge the DMA bring-up so real matmuls start at full
clock; o_proj of chunk qi is interleaved per-ki into chunk qi+1's attention so the
PE never stalls on the exp chain; softmax division reads PV numerators straight
from PSUM; score matmuls and exp are trimmed to live columns on the causal diagonal.
"""

import sys

sys.path.insert(0, "/opt/trn_rl_repo")

from contextlib import ExitStack

import numpy as np
import ml_dtypes

import concourse.bass as bass
import concourse.mybir as mybir
import concourse.tile as tile
from concourse import bacc

FP = mybir.dt.float32
BF = mybir.dt.bfloat16
EXP = mybir.ActivationFunctionType.Exp

B, S, HID = 2, 2048, 1024
H, D = 16, 64
QC = 512
KT = 128
NQC = S // QC
NKT = S // KT
KHID = HID // 128
N_DUMMY = 11


def build_program():
    nc = bacc.Bacc("TRN2", target_bir_lowering=False, debug=False, num_devices=8, num_swdge_queues=2)

    wqkT = nc.dram_tensor("wqkT", [128, KHID * 512], BF, kind="ExternalInput").ap()
    hsbT = nc.dram_tensor("hsbT", [128, NQC * KHID * QC], BF, kind="ExternalInput").ap()
    wvT = nc.dram_tensor("wvT", [128, KHID * 256], BF, kind="ExternalInput").ap()
    woT = nc.dram_tensor("woT", [128, 2 * HID], BF, kind="ExternalInput").ap()
    cos2T = nc.dram_tensor("cos2T", [64, S], BF, kind="ExternalInput").ap()
    ssin2T = nc.dram_tensor("ssin2T", [64, S], BF, kind="ExternalInput").ap()
    maskD = nc.dram_tensor("maskD", [128, 256], BF, kind="ExternalInput").ap()
    pmat = nc.dram_tensor("pmat", [128, 128], BF, kind="ExternalInput").ap()
    outT = nc.dram_tensor("outT", [128, NQC * 8 * QC], BF, kind="ExternalOutput").ap()

    with tile.TileContext(nc) as tc:
        build_tile_program(tc, wqkT, hsbT, wvT, woT, cos2T, ssin2T, maskD, pmat, outT)
    nc.compile()
    return nc


def build_tile_program(tc, wqkT, hsbT, wvT, woT, cos2T, ssin2T, maskD, pmat, outT):
    nc = tc.nc
    with ExitStack() as ctx:
        const = ctx.enter_context(tc.tile_pool(name="const", bufs=1))
        persist = ctx.enter_context(tc.tile_pool(name="persist", bufs=1))
        work = ctx.enter_context(tc.tile_pool(name="work", bufs=5))
        owp = ctx.enter_context(tc.tile_pool(name="owp", bufs=2))
        expp = ctx.enter_context(tc.tile_pool(name="expp", bufs=8))
        small = ctx.enter_context(tc.tile_pool(name="small", bufs=3))
        ps_main = ctx.enter_context(tc.tile_pool(name="ps_main", bufs=2, space="PSUM"))
        ps_sc = ctx.enter_context(tc.tile_pool(name="ps_sc", bufs=2, space="PSUM"))

        # ---- PE warmup scratch (no DMA dependency) ----
        dum_l = const.tile([128, 128], BF, name="dum_l")
        dum_r = const.tile([128, QC], BF, name="dum_r")
        nc.gpsimd.memset(dum_l[:], 0.0)
        nc.vector.memset(dum_r[:], 0.0)

        # ---- input DMAs, in three dependency-gated stages so the transfers
        # needed first get the full DMA bandwidth instead of a fair share ----
        wqk_sb = const.tile([128, KHID, 512], BF, name="wqk_sb")
        hsb_sb = const.tile([128, NQC, KHID, QC], BF, name="hsb_sb")
        cos_sb = const.tile([128, S], BF, name="cos_sb")
        ssin_sb = const.tile([128, S], BF, name="ssin_sb")
        pmat_sb = const.tile([128, 128], BF, name="pmat_sb")
        tri_sb = const.tile([128, 2, 128], BF, name="tri_sb")
        wv_sb = const.tile([128, KHID, 256], BF, name="wv_sb")
        wo_sb = const.tile([128, 2, HID], BF, name="wo_sb")

        wqkR = wqkT.rearrange("p (k m) -> p k m", k=KHID)

        def load_wqk_rb(rb, eng):
            eng.dma_start(
                wqk_sb[:, :, rb * 128:(rb + 1) * 128], wqkR[:, :, rb * 128:(rb + 1) * 128]
            )

        def load_hsb(t, eng):
            eng.dma_start(
                hsb_sb[:, t],
                hsbT[:, t * KHID * QC:(t + 1) * KHID * QC].rearrange("p (k s) -> p k s", k=KHID),
            )

        # stage 0: everything the first proj/rope of chunk 0 (pair0) touches,
        # split fine so the descriptors spread across many DMA queues
        load_wqk_rb(0, nc.sync)
        for kk in range(4):
            eng = nc.sync if kk % 2 == 0 else nc.scalar
            eng.dma_start(
                hsb_sb[:, 0, 2 * kk:2 * kk + 2, :],
                hsbT[:, 2 * kk * QC:(2 * kk + 2) * QC].rearrange("p (k s) -> p k s", k=2),
            )
        nc.scalar.dma_start(cos_sb[0:64, :], cos2T[:])
        nc.sync.dma_start(ssin_sb[0:64, :], ssin2T[:])
        nc.scalar.dma_start(pmat_sb[:], pmat[:])
        load_wqk_rb(2, nc.scalar)

        # stage gates: 2-byte copies whose WAW deps hold the next stage's DMAs
        # back until the anchor transfer of the prior stage has completed
        anchor0 = hsb_sb[0:1, 0, 7, 0:1]
        for dst in (wqk_sb[0:1, 0, 128:129], wqk_sb[0:1, 0, 384:385],
                    wv_sb[0:1, 0, 0:1], tri_sb[0:1, 0, 0:1],
                    hsb_sb[0:1, 1, 0, 0:1]):
            nc.vector.tensor_copy(dst, anchor0)
        # stage 1: pair1 weights, V weights, mask, chunk 1
        load_wqk_rb(1, nc.sync)
        for kk in range(2):
            eng = nc.sync if kk == 0 else nc.scalar
            eng.dma_start(
                hsb_sb[:, 1, 4 * kk:4 * kk + 4, :],
                hsbT[:, (8 + 4 * kk) * QC:(8 + 4 * kk + 4) * QC].rearrange(
                    "p (k s) -> p k s", k=4),
            )
        nc.scalar.dma_start(wv_sb[:], wvT.rearrange("p (k m) -> p k m", k=KHID))
        nc.sync.dma_start(tri_sb[:], maskD.rearrange("p (r c) -> p r c", r=2))
        load_wqk_rb(3, nc.scalar)

        anchor1 = hsb_sb[0:1, 1, 7, 0:1]
        for dst in (wo_sb[0:1, 0, 0:1], hsb_sb[0:1, 2, 0, 0:1],
                    hsb_sb[0:1, 3, 0, 0:1]):
            nc.vector.tensor_copy(dst, anchor1)
        # stage 2: o_proj weights, chunks 2-3
        nc.scalar.dma_start(wo_sb[:], woT.rearrange("p (k m) -> p k m", k=2))
        load_hsb(2, nc.sync)
        load_hsb(3, nc.scalar)

        nc.vector.tensor_copy(cos_sb[64:128, :], cos_sb[0:64, :])
        nc.vector.tensor_copy(ssin_sb[64:128, :], ssin_sb[0:64, :])

        # per-chunk tiles: avoids conservative whole-tile WAR/WAW deps between
        # writers of one chunk and readers of another
        qk_q = [persist.tile([128, 4, QC], BF, name=f"qk_q{t}") for t in range(NQC)]
        v_q = [persist.tile([128, 4, 4 * 65], BF, name=f"v_q{t}") for t in range(NQC)]
        att_q = [persist.tile([128, 2, QC], BF, name=f"att_q{t}") for t in range(NQC)]
        for t in range(NQC):
            nc.gpsimd.memset(
                v_q[t].rearrange("p t (h c) -> p t h c", c=65)[:, :, :, 64:65], 1.0
            )

        # ---- PE warmup: keep the array streaming until real inputs land ----
        for _ in range(N_DUMMY):
            psd = ps_sc.tile([128, QC], FP, name="psd", tag="sc")
            nc.tensor.matmul(psd[:], dum_l[:], dum_r[:], start=True, stop=True)

        def proj_mm(rb, t):
            """bf16 Q/K projection matmuls; returns (psum tile, x bf16 copy)."""
            ps = ps_main.tile([128, QC], FP, name="ps_qk", tag="ps")
            for kk in range(KHID):
                nc.tensor.matmul(
                    ps[:],
                    wqk_sb[:, kk, rb * 128:(rb + 1) * 128],
                    hsb_sb[:, t, kk, :],
                    start=(kk == 0),
                    stop=(kk == KHID - 1),
                )
            x = work.tile([128, QC], BF, name="x_rope", tag="xrope")
            nc.scalar.copy(x[:], ps[:])
            return ps, x

        def rope_mm(x):
            xs_ps = ps_main.tile([128, QC], FP, name="xs_ps", tag="ps")
            nc.tensor.matmul(xs_ps[:], pmat_sb[:], x[:], start=True, stop=True)
            return xs_ps

        def rope_fin(rb, t, x, xs_ps):
            csl = slice(t * QC, (t + 1) * QC)
            t1 = work.tile([128, QC], BF, name="t1_rope", tag="t1rope")
            t2 = work.tile([128, QC], BF, name="t2_rope", tag="t2rope")
            nc.vector.tensor_mul(t1[:], x[:], cos_sb[:, csl])
            nc.vector.tensor_mul(t2[:], xs_ps[:], ssin_sb[:, csl])
            nc.vector.tensor_add(qk_q[t][:, rb, :], t1[:], t2[:])

        def v_proj(tt):
            psv = ps_main.tile([128, 256], FP, name="ps_v", tag="ps")
            for kk in range(KHID):
                nc.tensor.matmul(
                    psv[:],
                    hsb_sb[:, tt // 4, kk, (tt % 4) * 128:(tt % 4 + 1) * 128],
                    wv_sb[:, kk, :],
                    start=(kk == 0),
                    stop=(kk == KHID - 1),
                )
            nc.vector.tensor_copy(
                v_q[tt // 4][:, tt % 4, :].rearrange("p (h c) -> p h c", c=65)[:, :, 0:64],
                psv[:].rearrange("p (h c) -> p h c", c=64),
            )

        def proj_block(t):
            """Full projection of chunk t: Q/K (pair0 rbs first) + V."""
            parts = {}
            for rb in (0, 2, 1, 3):
                parts[rb] = proj_mm(rb, t)
            for rb in (0, 2, 1, 3):
                ps, x = parts[rb]
                xs_ps = rope_mm(x)
                rope_fin(rb, t, x, xs_ps)
            for tt in range(4 * t, 4 * t + 4):
                v_proj(tt)

        def attention_unit(pair, qi, l_pair, fillers, max_pops=99, l_on_act=False):
            """scores^T -> exp -> PV for heads (2*pair, 2*pair+1); po kept in PSUM."""
            nki = 4 * qi + 4
            pops = 0
            po0 = ps_main.tile([65, QC], FP, name="po0", tag="po", bufs=2)
            po1 = ps_main.tile([65, QC], FP, name="po1", tag="po", bufs=2)
            for ki in range(nki):
                kq = qk_q[ki // 4]
                ksl = slice((ki % 4) * KT, (ki % 4 + 1) * KT)
                j = ki - 4 * qi
                lo = 0 if j < 0 else 128 * j  # first live q column in this chunk
                psc = ps_sc.tile([128, 2, QC], FP, name="psc", tag="sc")
                nc.tensor.matmul(
                    psc[:, 0, lo:QC], kq[0:64, 2 + pair, ksl], qk_q[qi][0:64, pair, lo:QC],
                    start=True, stop=True,
                )
                nc.tensor.matmul(
                    psc[:, 1, lo:QC], kq[64:128, 2 + pair, ksl], qk_q[qi][64:128, pair, lo:QC],
                    start=True, stop=True,
                )
                e = expp.tile([128, 2, QC], BF, name="e", tag="exp")
                nc.scalar.activation(
                    e[:, :, lo:QC], psc[:, :, lo:QC], EXP, scale=0.125
                )
                if j >= 0:
                    nc.vector.tensor_mul(
                        e[:, :, lo:lo + 128], e[:, :, lo:lo + 128], tri_sb[:]
                    )
                h0 = 2 * pair
                h1 = 2 * pair + 1
                vq = v_q[ki // 4]
                ks = ki % 4
                nc.tensor.matmul(
                    po0[:, lo:QC], vq[:, ks, h0 * 65:(h0 + 1) * 65], e[:, 0, lo:QC],
                    start=(ki == 0), stop=(ki == nki - 1),
                )
                nc.tensor.matmul(
                    po1[:, lo:QC], vq[:, ks, h1 * 65:(h1 + 1) * 65], e[:, 1, lo:QC],
                    start=(ki == 0), stop=(ki == nki - 1),
                )
                if (fillers and pops < max_pops
                        and (qi < 2 or ki % 2 == 0) and ki < nki - 2):
                    fillers.pop(0)()
                    pops += 1
            l_eng = nc.scalar.copy if l_on_act else nc.vector.tensor_copy
            l_eng(l_pair[0:1, :], po0[64:65, :])
            l_eng(l_pair[32:33, :], po1[64:65, :])
            return po0, po1

        def division(pair, qi, l_pair, po0, po1):
            rl = small.tile([64, QC], FP, name="rl", tag="rl", bufs=6)
            scr = small.tile([64, QC], FP, name="scr", tag="scr", bufs=3)
            nc.vector.reciprocal_approx_accurate(out=rl[:], in_=l_pair[:], scratch=scr[:])
            for sub, po in enumerate([po0, po1]):
                if sub == 0:
                    src = rl[0:1, :]
                else:
                    rlrow = small.tile([1, QC], FP, name="rlrow", tag="rlrow", bufs=4)
                    nc.vector.tensor_copy(rlrow[:], rl[32:33, :])
                    src = rlrow[:]
                rb_ = small.tile([64, QC], FP, name="rb_", tag="rbb", bufs=6)
                nc.gpsimd.partition_broadcast(rb_[:], src)
                nc.vector.tensor_mul(
                    att_q[qi][sub * 64:(sub + 1) * 64, pair, :], po[0:64, :], rb_[:]
                )

        def oproj_units(qi, tail=False):
            """Eight filler units (one per output row block) for o_proj of chunk qi."""

            def mk(ot):
                def f():
                    ow = owp.tile([128, QC], BF, name="ow", tag="ow", bufs=4)
                    pw = ps_main.tile([128, QC], FP, name="pw", tag="ps")
                    for p in range(2):
                        nc.tensor.matmul(
                            pw[:],
                            wo_sb[:, p, ot * 128:(ot + 1) * 128],
                            att_q[qi][:, p, :],
                            start=(p == 0),
                            stop=(p == 1),
                        )
                    if tail and ot % 2 == 1:
                        nc.scalar.copy(ow[:], pw[:])
                    else:
                        nc.vector.tensor_copy(ow[:], pw[:])
                    off = (qi * 8 + ot) * QC
                    nc.sync.dma_start(outT[:, off:off + QC], ow[:])
                return f

            return [mk(ot) for ot in range(8)]

        # ---- emission ----
        proj_block(0)
        fillers = []
        for qi in range(NQC):
            last = qi + 1 == NQC
            pops = 2 if last else 99
            l0 = small.tile([64, QC], FP, name="l0", tag="lt", bufs=4)
            nc.gpsimd.memset(l0[:], 1.0)
            pa = attention_unit(0, qi, l0, fillers, max_pops=pops)
            division(0, qi, l0, *pa)
            l1 = small.tile([64, QC], FP, name="l1", tag="lt", bufs=4)
            nc.gpsimd.memset(l1[:], 1.0)
            pb = attention_unit(1, qi, l1, fillers, max_pops=pops, l_on_act=last)
            if not last:
                proj_block(qi + 1)
                division(1, qi, l1, *pb)
            else:
                # division first so its DVE chain runs under the drained
                # oproj(qi-1) matmuls, then the final chunk's o_proj
                division(1, qi, l1, *pb)
                for f in fillers:
                    f()
            fillers = oproj_units(qi, tail=last)
        for f in fillers:
            f()


# ---------- host-side shard preparation ----------

def make_core_inputs(hidden_states, cos, sin, w_qkv, w_o):
    """Returns list of 8 in_maps (numpy, bf16 where needed)."""
    bf = ml_dtypes.bfloat16
    hs = np.asarray(hidden_states, np.float32)
    cos = np.asarray(cos, np.float32)
    sin = np.asarray(sin, np.float32)
    w_qkv = np.asarray(w_qkv, np.float32)
    w_o = np.asarray(w_o, np.float32)

    cos2T = np.ascontiguousarray(cos.T).astype(bf)
    ssin2T = np.ascontiguousarray(sin.T).astype(bf)
    # signed rotate-half permutation: out[m] = sign(m) * x[partner(m)]
    # lhsT layout: pmat[k, m] = sign(m) at k = partner(m)
    pmat = np.zeros((128, 128), np.float32)
    for m in range(128):
        d = m % 64
        base = m - d
        if d < 32:
            pmat[base + d + 32, m] = -1.0
        else:
            pmat[base + d - 32, m] = 1.0
    pmat = pmat.astype(bf)

    kp = np.arange(128)[:, None]
    cc = np.arange(128)[None, :]
    tri = (kp <= cc).astype(bf)
    maskD = np.concatenate([tri, tri], axis=1)

    def swz(a):
        # [K*128, M] -> [128, K*M] partition-major (matches SBUF tiles)
        k = a.shape[0] // 128
        return np.ascontiguousarray(
            a.reshape(k, 128, a.shape[1]).transpose(1, 0, 2).reshape(128, -1))

    in_maps = []
    for c in range(8):
        b, g = divmod(c, 4)
        heads = range(4 * g, 4 * g + 4)
        hsT = np.ascontiguousarray(hs[b].T).astype(bf)  # [HID, S]
        # chunk-major swizzle: [128, t, kk, s']
        hsb = np.ascontiguousarray(
            hsT.reshape(8, 128, 4, 512).transpose(1, 2, 0, 3).reshape(128, -1))
        wq = np.concatenate([w_qkv[h * 64:(h + 1) * 64] for h in heads], 0)
        wk = np.concatenate([w_qkv[HID + h * 64:HID + (h + 1) * 64] for h in heads], 0)
        wv = np.concatenate([w_qkv[2 * HID + h * 64:2 * HID + (h + 1) * 64] for h in heads], 0)
        wqkT = swz(np.ascontiguousarray(np.concatenate([wq, wk], 0).T).astype(bf))
        wvT = swz(np.ascontiguousarray(wv.T).astype(bf))
        woT = swz(np.ascontiguousarray(
            np.concatenate([w_o[:, h * 64:(h + 1) * 64] for h in heads], 1).T
        ).astype(bf))
        in_maps.append({
            "hsbT": hsb, "wqkT": wqkT, "wvT": wvT, "woT": woT,
            "cos2T": cos2T, "ssin2T": ssin2T, "maskD": maskD, "pmat": pmat,
        })
    return in_maps


def unswizzle_out(o2):
    # [128, qi*half*oi*512] -> outT [1024, 2048]
    a = o2.reshape(128, NQC, 2, 4, QC)
    return np.ascontiguousarray(
        a.transpose(2, 3, 0, 1, 4).reshape(HID, S))


# ---------- standalone kernel entry ----------

from concourse.bass_utils import run_bass_kernel_spmd

_CACHED_NC = None


def get_program():
    global _CACHED_NC
    if _CACHED_NC is None:
        _CACHED_NC = build_program()
    return _CACHED_NC


def run(inputs, trace=False):
    nc = get_program()
    in_maps = make_core_inputs(**inputs)
    res = run_bass_kernel_spmd(nc, in_maps, core_ids=list(range(8)), trace=trace)
    out = np.zeros((B, S, HID), np.float32)
    for c, r in enumerate(res.results):
        out[c // 4] += unswizzle_out(r["outT"]).T.astype(np.float32)
    return out, res


def kernel(**inputs):
    out, _ = run(inputs, trace=False)
    return out
